# revision 10
# baseline (speedup 1.0000x reference)
"""DISCO downsample conv (3x3, stride 2, pad 1) on 8 Trainium2 NeuronCores.

Strategy:
  - Effective weights w[o,i,kh,kw] = sum_b coeff[o,i,b]*basis[b,kh,kw] are tiny:
    computed on host, shipped per-tap transposed as wt[i, tap, o] (fp16),
    pre-scaled by 1/XSCALE to undo the input scaling for free.
  - x is zero-padded (H+2, W+2) on host, W phase-split into [even | odd]
    columns so every conv tap reads a contiguous run of 256 columns, scaled
    by XSCALE=4 (centers N(0,1) data in fp8-e3m4's narrow normal range) and
    cast to fp8 e3m4 (4-bit mantissa, rel err ~1.26e-2 on this conv). The
    matmul streams the e3m4 moving operand at the same 1 row/cycle as fp16
    while the stationary weights stay fp16, so TensorE time is unchanged but
    input HBM traffic halves -- the DMA engines drop well below the TensorE
    streaming floor (64 row-pairs x 9 taps x 512 px @ 2.4GHz = ~125us),
    which is the binding constraint for this shape.
    (fp8 DoubleRow tap-pairing was tried and reverted: a DoubleRow matmul
    following any normal-mode matmul wedges the PE -- hardware hang -- and
    with the 2e-2 accuracy gate nothing cheaper than 9 fp16-rate tap matmuls
    per row-pair survives.)
  - Sharding: 8 shards = (batch b in 0..3) x (H half in 0..1). Each core gets
    padded rows [256*h, 256*h + 257) of batch b -- the 1-row halo is part of
    the shard, so no inter-core communication is needed.
  - Startup: no DMA byte can land before ~9us (engine preamble ~6us + queue
    arm ~3us), so TensorE runs dummy matmuls on a memset tile in that dead
    window to pre-warm the HAM clock gate; without it the first ~6us of real
    matmuls run at half clock. The dummy chain is sized to end right as the
    first x chunk lands -- a gap between dummies and real work resets the
    ramp.
  - Per core: 4 row-blocks (32 output rows each = 65 input rows), block 0
    loaded in fine chunks so compute starts as soon as the first rows land,
    later blocks in 2 big chunks (per-dma_start ~2us re-arm amortized). For
    each pair of output rows: one PSUM bank [96, 512], 9 accumulating
    matmuls, then ScalarE adds bias while copying PSUM -> SBUF (fp16), and
    rows flush to HBM on the ACT HWDGE ring (kept warm all kernel; the idle
    SP ring would pay ~2us re-arm per flush). The last block's final two
    rows become single-row N=256 tiles, halving the end-of-kernel ACT+flush
    chain.
"""

import os
import sys
import types

import numpy as np


# ----------------------------------------------------------------------------
# Environment bootstrap (self-contained: no reads from /root/problem).
# ----------------------------------------------------------------------------
def _ensure_paths():
    for p in (
        "/root/.axon_site",
        "/root/.axon_site/_ro/trn_rl_repo",
        "/root/.axon_site/_ro/pypackages",
        "/opt/trn_rl_repo",
    ):
        if os.path.isdir(p) and p not in sys.path:
            sys.path.append(p)


_ensure_paths()

import ml_dtypes  # noqa: E402


def _install_ntff_hook():
    """Register the NTFF profile hook (used when tracing; harmless otherwise)."""
    try:
        import antenv
    except ImportError:
        return
    if "antenv.axon_hooks" not in sys.modules:
        hooks_mod = types.ModuleType("antenv.axon_hooks")
        _hook = [None]
        hooks_mod.set_axon_ntff_profile_hook = lambda h: _hook.__setitem__(0, h)
        hooks_mod.get_axon_ntff_profile_hook = lambda: _hook[0]
        sys.modules["antenv.axon_hooks"] = hooks_mod
        antenv.axon_hooks = hooks_mod
    from antenv.axon_hooks import (
        get_axon_ntff_profile_hook,
        set_axon_ntff_profile_hook,
    )

    if get_axon_ntff_profile_hook() is None:
        try:
            from trn_agent_boot.trn_boot import _ntff_profile_via_ctypes

            so = "/opt/axon/libaxon_pjrt.so"
            if os.path.exists(so):
                set_axon_ntff_profile_hook(_ntff_profile_via_ctypes(so))
        except Exception:
            pass


_install_ntff_hook()

import concourse.bass as bass  # noqa: E402
import concourse.tile as tile  # noqa: E402
from concourse import bacc, mybir  # noqa: E402
import concourse.bass_utils as _bu  # noqa: E402

# Artifact upload needs a bucket that isn't reachable here; keep traces local.
_bu.upload_artifacts = lambda tmpdir: f"local:{tmpdir}"

XDT = mybir.dt.float8e3     # moving operand: fp8 e3m4 (4-bit mantissa)
WDT = mybir.dt.float16      # stationary weights stay fp16
F16 = mybir.dt.float16
F32 = mybir.dt.float32
NP_XDT = ml_dtypes.float8_e3m4
XSCALE = 4.0                # x *= 4 fits N(0,1) in e3m4 normals; w /= 4
XCLIP = 15.5                # e3m4 max finite

C = 96          # channels (in == out)
K = 3           # kernel size
N_CORES = 8
H = W = 512     # input spatial
HO = WO = 256   # output spatial
HP = H + 2      # padded rows
WP = W + 2      # padded cols (phase-split: [257 even | 257 odd])
SH_ROWS = 257   # padded rows per shard (256 + 1 halo)
CORE_HO = 128   # output rows per core
BH = 32         # output rows per block
NBLK = CORE_HO // BH
IN_ROWS = 2 * BH + 1  # input rows per block (65)
N_WARM = 13     # HAM pre-warm dummy matmuls

# column base per kw tap: even-phase col 2*ow -> slot ow (base 0);
# odd-phase col 2*ow+1 -> slot 257+ow; even col 2*ow+2 -> slot ow+1.
_KW_BASE = {0: 0, 1: 257, 2: 1}

_PROGRAM_CACHE = {}


def _build_program():
    """One SPMD Bass program, shared by all 8 cores."""
    nc = bacc.Bacc()
    x_d = nc.dram_tensor("x", [C, SH_ROWS, WP], XDT, kind="ExternalInput")
    w_d = nc.dram_tensor("wt", [C, K * K, C], WDT, kind="ExternalInput")
    b_d = nc.dram_tensor("bias", [C, 1], F32, kind="ExternalInput")
    y_d = nc.dram_tensor("out", [C, CORE_HO, WO], F16, kind="ExternalOutput")

    with tile.TileContext(nc) as tc:
        with (
            tc.tile_pool(name="const", bufs=1) as cpool,
            tc.tile_pool(name="xin", bufs=2) as xpool,
            tc.tile_pool(name="oout", bufs=2) as opool,
            tc.tile_pool(name="psum", bufs=8, space=bass.MemorySpace.PSUM) as ppool,
        ):
            # HAM pre-warm source: a zeroed fp16 tile, matmul'd into junk PSUM
            # while the first x chunk is still in DMA-queue-arm limbo.
            warm = cpool.tile([C, 352], F16)
            nc.vector.memset(warm[:], 0.0)
            # constants ride SWDGE: it emits right after the engine preamble
            # (~6us) and lands the small wt before the first x chunk arrives
            wt = cpool.tile([C, K * K, C], WDT)
            nc.gpsimd.dma_start(wt[:], w_d[:])
            bias = cpool.tile([C, 1], F32)
            nc.gpsimd.dma_start(bias[:], b_d[:])

            # warm-up matmuls use full-size PSUM tiles so they don't perturb
            # the pool's bank packing for the real accumulators
            for i in range(N_WARM):
                wps = ppool.tile([C, 2 * WO], F32, name="wps", tag="ps")
                nc.tensor.matmul(
                    wps[:, 0:256], warm[:, 0:C], warm[:, C : C + 256],
                    start=True, stop=True,
                )

            for blk in range(NBLK):
                xt = xpool.tile([C, IN_ROWS, WP], XDT, name="xt", tag="xt")
                r0 = 2 * BH * blk
                if blk == 0:
                    # first 5 rows split across BOTH HWDGE rings (they arm in
                    # parallel; a single 5-row chunk completes ~1.5us later)
                    nc.sync.dma_start(xt[:, 0:3, :], x_d[:, 0:3, :])
                    nc.scalar.dma_start(xt[:, 3:5, :], x_d[:, 3:5, :])
                    chunks, rr = (13, 15, 16, 16), 5
                else:
                    chunks, rr = (33, 32), 0
                for nrows in chunks:
                    nc.sync.dma_start(
                        xt[:, rr : rr + nrows, :],
                        x_d[:, r0 + rr : r0 + rr + nrows, :],
                    )
                    rr += nrows
                assert rr == IN_ROWS
                out_sb = opool.tile([C, BH, WO], F16)
                last = blk == NBLK - 1
                # last block: final two rows become single-row N=256 tiles
                # below, halving the end-of-kernel ACT+flush chain
                for t in range(BH // 2 - 1 if last else BH // 2):
                    ps = ppool.tile([C, 2 * WO], F32, name="ps", tag="ps")
                    for tap in range(K * K):
                        kh, kw = tap // K, tap % K
                        cb = _KW_BASE[kw]
                        rhs = xt[:, 4 * t + kh : 4 * t + kh + 3 : 2, cb : cb + WO]
                        nc.tensor.matmul(
                            ps[:],
                            wt[:, tap, :],
                            rhs,
                            start=(tap == 0),
                            stop=(tap == K * K - 1),
                        )
                    nc.scalar.activation(
                        out_sb[:, 2 * t : 2 * t + 2, :],
                        ps[:].rearrange("p (a b) -> p a b", a=2),
                        mybir.ActivationFunctionType.Identity,
                        bias=bias[:],
                    )
                    # flush finished rows on the ACT HWDGE ring as soon as they
                    # complete; the last block flushes in quarters to shrink
                    # the kernel tail
                    flush_at = (
                        (7, 11, 13, 14) if last else (BH // 4 - 1, BH // 2 - 1)
                    )
                    if t in flush_at:
                        fi = flush_at.index(t)
                        prev = 0 if fi == 0 else (flush_at[fi - 1] + 1)
                        lo, hi = 2 * prev, 2 * t + 2
                        nc.scalar.dma_start(
                            y_d[:, BH * blk + lo : BH * blk + hi, :],
                            out_sb[:, lo:hi, :],
                        )
                if last:
                    for r in (BH - 2, BH - 1):
                        psr = ppool.tile([C, WO], F32, name="psr", tag="ps")
                        for tap in range(K * K):
                            kh, kw = tap // K, tap % K
                            cb = _KW_BASE[kw]
                            nc.tensor.matmul(
                                psr[:],
                                wt[:, tap, :],
                                xt[:, 2 * r + kh, cb : cb + WO],
                                start=(tap == 0),
                                stop=(tap == K * K - 1),
                            )
                        nc.scalar.activation(
                            out_sb[:, r, :],
                            psr[:],
                            mybir.ActivationFunctionType.Identity,
                            bias=bias[:],
                        )
                        nc.scalar.dma_start(
                            y_d[:, BH * blk + r, :], out_sb[:, r, :]
                        )

    nc.compile()
    return nc


def _get_program():
    if "nc" not in _PROGRAM_CACHE:
        _PROGRAM_CACHE["nc"] = _build_program()
    return _PROGRAM_CACHE["nc"]


def _prepare_inputs(x, coeff, basis, bias):
    """Host prep: effective weights, padded phase-split e3m4 x, shards."""
    x = np.asarray(x)
    coeff = np.asarray(coeff)
    basis = np.asarray(basis)
    bias = np.asarray(bias)
    B = coeff.shape[2]
    # wt[i, tap, o] = sum_b coeff[o,i,b] * basis[b, tap], pre-scaled 1/XSCALE
    w_eff = (
        coeff.astype(np.float32).reshape(C * C, B)
        @ basis.astype(np.float32).reshape(B, K * K)
    ).reshape(C, C, K * K)
    wt = np.ascontiguousarray(
        w_eff.transpose(1, 2, 0) * np.float32(1.0 / XSCALE)
    ).astype(np.float16)

    xs = np.clip(x.astype(np.float32) * np.float32(XSCALE), -XCLIP, XCLIP)
    xb = xs.astype(NP_XDT)
    xph = np.zeros((x.shape[0], C, HP, WP), dtype=NP_XDT)
    # even phase: padded col 2j -> orig col 2j-1  (slot j=1..256)
    xph[:, :, 1 : H + 1, 1:257] = xb[:, :, :, 1::2]
    # odd phase: padded col 2j+1 -> orig col 2j  (slot 257+j, j=0..255)
    xph[:, :, 1 : H + 1, 257:513] = xb[:, :, :, 0::2]

    bias2 = np.ascontiguousarray(bias.astype(np.float32).reshape(C, 1))

    in_maps = []
    for s in range(N_CORES):
        b_idx, h_idx = divmod(s, 2)
        shard = np.ascontiguousarray(
            xph[b_idx, :, 256 * h_idx : 256 * h_idx + SH_ROWS, :]
        )
        in_maps.append({"x": shard, "wt": wt, "bias": bias2})
    return in_maps


def _assemble(results, n_batch):
    out = np.empty((n_batch, C, 2 * CORE_HO, WO), dtype=np.float32)
    for s in range(N_CORES):
        b_idx, h_idx = divmod(s, 2)
        out[b_idx, :, CORE_HO * h_idx : CORE_HO * (h_idx + 1), :] = results[s][
            "out"
        ].astype(np.float32)
    return out


def run(x, coeff, basis, bias, trace=False, trace_cores=None):
    """Run the kernel; returns (full_output, BassKernelResults)."""
    nc = _get_program()
    in_maps = _prepare_inputs(x, coeff, basis, bias)
    last_err = None
    for attempt in range(3):
        try:
            res = _bu.run_bass_kernel_spmd(
                nc,
                in_maps,
                list(range(N_CORES)),
                trace=trace,
                trace_cores=trace_cores,
            )
            return _assemble(res.results, x.shape[0]), res
        except Exception as e:  # transient NRT device-unrecoverable after
            last_err = e        # abrupt neighbor-process exits; nudge + retry
            if attempt == 2 or "UNAVAILABLE" not in str(e):
                raise
            import time

            import jax
            import jax.numpy as jnp

            time.sleep(15)
            try:
                a = jnp.ones((8, 8))
                (a @ a).block_until_ready()
            except Exception:
                time.sleep(15)
    raise last_err


def kernel(x, coeff, basis, bias):
    out, _ = run(x, coeff, basis, bias, trace=False)
    return out


# revision 11
# speedup vs baseline: 1.0168x; 1.0168x over previous
"""DISCO downsample conv (3x3, stride 2, pad 1) on 8 Trainium2 NeuronCores.

Strategy:
  - Effective weights w[o,i,kh,kw] = sum_b coeff[o,i,b]*basis[b,kh,kw] are tiny:
    computed on host, shipped per-tap transposed as wt[i, tap, o] (fp16),
    pre-scaled by 1/XSCALE to undo the input scaling for free.
  - x is zero-padded (H+2, W+2) on host, W phase-split into [even | odd]
    columns so every conv tap reads a contiguous run of 256 columns, scaled
    by XSCALE=4 (centers N(0,1) data in fp8-e3m4's narrow normal range) and
    cast to fp8 e3m4 (4-bit mantissa, rel err ~1.26e-2 on this conv). The
    matmul streams the e3m4 moving operand at the same 1 row/cycle as fp16
    while the stationary weights stay fp16, so TensorE time is unchanged but
    input HBM traffic halves -- the DMA engines drop well below the TensorE
    streaming floor (64 row-pairs x 9 taps x 512 px @ 2.4GHz = ~125us),
    which is the binding constraint for this shape.
    (fp8 DoubleRow tap-pairing was tried and reverted: a DoubleRow matmul
    following any normal-mode matmul wedges the PE -- hardware hang -- and
    with the 2e-2 accuracy gate nothing cheaper than 9 fp16-rate tap matmuls
    per row-pair survives.)
  - Sharding: 8 shards = (batch b in 0..3) x (H half in 0..1). Each core gets
    padded rows [256*h, 256*h + 257) of batch b -- the 1-row halo is part of
    the shard, so no inter-core communication is needed.
  - Startup: no DMA byte can land before ~9us (engine preamble ~6us + queue
    arm ~3us), so TensorE runs dummy matmuls on a memset tile in that dead
    window to pre-warm the HAM clock gate; without it the first ~6us of real
    matmuls run at half clock. The dummy chain is sized to end right as the
    first x chunk lands -- a gap between dummies and real work resets the
    ramp.
  - Per core: 4 row-blocks (32 output rows each = 65 input rows), block 0
    loaded in fine chunks so compute starts as soon as the first rows land,
    later blocks in 2 big chunks (per-dma_start ~2us re-arm amortized). For
    each pair of output rows: one PSUM bank [96, 512], 9 accumulating
    matmuls, then ScalarE adds bias while copying PSUM -> SBUF (fp16), and
    rows flush to HBM on the ACT HWDGE ring (kept warm all kernel; the idle
    SP ring would pay ~2us re-arm per flush). The last block's final two
    rows become single-row N=256 tiles, halving the end-of-kernel ACT+flush
    chain.
"""

import os
import sys
import types

import numpy as np


# ----------------------------------------------------------------------------
# Environment bootstrap (self-contained: no reads from /root/problem).
# ----------------------------------------------------------------------------
def _ensure_paths():
    for p in (
        "/root/.axon_site",
        "/root/.axon_site/_ro/trn_rl_repo",
        "/root/.axon_site/_ro/pypackages",
        "/opt/trn_rl_repo",
    ):
        if os.path.isdir(p) and p not in sys.path:
            sys.path.append(p)


_ensure_paths()

import ml_dtypes  # noqa: E402


def _install_ntff_hook():
    """Register the NTFF profile hook (used when tracing; harmless otherwise)."""
    try:
        import antenv
    except ImportError:
        return
    if "antenv.axon_hooks" not in sys.modules:
        hooks_mod = types.ModuleType("antenv.axon_hooks")
        _hook = [None]
        hooks_mod.set_axon_ntff_profile_hook = lambda h: _hook.__setitem__(0, h)
        hooks_mod.get_axon_ntff_profile_hook = lambda: _hook[0]
        sys.modules["antenv.axon_hooks"] = hooks_mod
        antenv.axon_hooks = hooks_mod
    from antenv.axon_hooks import (
        get_axon_ntff_profile_hook,
        set_axon_ntff_profile_hook,
    )

    if get_axon_ntff_profile_hook() is None:
        try:
            from trn_agent_boot.trn_boot import _ntff_profile_via_ctypes

            so = "/opt/axon/libaxon_pjrt.so"
            if os.path.exists(so):
                set_axon_ntff_profile_hook(_ntff_profile_via_ctypes(so))
        except Exception:
            pass


_install_ntff_hook()

import concourse.bass as bass  # noqa: E402
import concourse.tile as tile  # noqa: E402
from concourse import bacc, mybir  # noqa: E402
import concourse.bass_utils as _bu  # noqa: E402

# Artifact upload needs a bucket that isn't reachable here; keep traces local.
_bu.upload_artifacts = lambda tmpdir: f"local:{tmpdir}"

XDT = mybir.dt.float8e3     # moving operand: fp8 e3m4 (4-bit mantissa)
WDT = mybir.dt.float16      # stationary weights stay fp16
F16 = mybir.dt.float16
F32 = mybir.dt.float32
NP_XDT = ml_dtypes.float8_e3m4
XSCALE = 4.0                # x *= 4 fits N(0,1) in e3m4 normals; w /= 4
XCLIP = 15.5                # e3m4 max finite

C = 96          # channels (in == out)
K = 3           # kernel size
N_CORES = 8
H = W = 512     # input spatial
HO = WO = 256   # output spatial
HP = H + 2      # padded rows
WP = W + 2      # padded cols (phase-split: [257 even | 257 odd])
SH_ROWS = 257   # padded rows per shard (256 + 1 halo)
CORE_HO = 128   # output rows per core
BH = 32         # output rows per block
NBLK = CORE_HO // BH
IN_ROWS = 2 * BH + 1  # input rows per block (65)
N_WARM = 26     # HAM pre-warm dummy matmuls

# column base per kw tap: even-phase col 2*ow -> slot ow (base 0);
# odd-phase col 2*ow+1 -> slot 257+ow; even col 2*ow+2 -> slot ow+1.
_KW_BASE = {0: 0, 1: 257, 2: 1}

_PROGRAM_CACHE = {}


def _build_program():
    """One SPMD Bass program, shared by all 8 cores."""
    nc = bacc.Bacc()
    x_d = nc.dram_tensor("x", [C, SH_ROWS, WP], XDT, kind="ExternalInput")
    w_d = nc.dram_tensor("wt", [C, K * K, C], WDT, kind="ExternalInput")
    b_d = nc.dram_tensor("bias", [C, 1], F32, kind="ExternalInput")
    y_d = nc.dram_tensor("out", [C, CORE_HO, WO], F16, kind="ExternalOutput")

    with tile.TileContext(nc) as tc:
        with (
            tc.tile_pool(name="const", bufs=1) as cpool,
            tc.tile_pool(name="xin", bufs=2) as xpool,
            tc.tile_pool(name="oout", bufs=2) as opool,
            tc.tile_pool(name="psum", bufs=8, space=bass.MemorySpace.PSUM) as ppool,
        ):
            # HAM pre-warm source: a zeroed fp16 tile, matmul'd into junk PSUM
            # while the first x chunk is still in DMA-queue-arm limbo.
            warm = cpool.tile([C, 352], F16)
            nc.vector.memset(warm[:], 0.0)
            # constants ride SWDGE: it emits right after the engine preamble
            # (~6us) and lands the small wt before the first x chunk arrives
            wt = cpool.tile([C, K * K, C], WDT)
            nc.gpsimd.dma_start(wt[:], w_d[:])
            bias = cpool.tile([C, 1], F32)
            nc.gpsimd.dma_start(bias[:], b_d[:])

            # warm-up matmuls use full-size PSUM tiles so they don't perturb
            # the pool's bank packing for the real accumulators
            for i in range(N_WARM):
                wps = ppool.tile([C, 2 * WO], F32, name="wps", tag="ps")
                nc.tensor.matmul(
                    wps[:, 0:256], warm[:, 0:C], warm[:, C : C + 256],
                    start=True, stop=True,
                )

            for blk in range(NBLK):
                xt = xpool.tile([C, IN_ROWS, WP], XDT, name="xt", tag="xt")
                chunks = (5, 13, 15, 16, 16) if blk == 0 else (33, 32)
                r0 = 2 * BH * blk
                rr = 0
                for nrows in chunks:
                    nc.sync.dma_start(
                        xt[:, rr : rr + nrows, :],
                        x_d[:, r0 + rr : r0 + rr + nrows, :],
                    )
                    rr += nrows
                assert rr == IN_ROWS
                out_sb = opool.tile([C, BH, WO], F16)
                last = blk == NBLK - 1
                # last block: final two rows become single-row N=256 tiles
                # below, halving the end-of-kernel ACT+flush chain
                for t in range(BH // 2 - 1 if last else BH // 2):
                    ps = ppool.tile([C, 2 * WO], F32, name="ps", tag="ps")
                    for tap in range(K * K):
                        kh, kw = tap // K, tap % K
                        cb = _KW_BASE[kw]
                        rhs = xt[:, 4 * t + kh : 4 * t + kh + 3 : 2, cb : cb + WO]
                        nc.tensor.matmul(
                            ps[:],
                            wt[:, tap, :],
                            rhs,
                            start=(tap == 0),
                            stop=(tap == K * K - 1),
                        )
                    nc.scalar.activation(
                        out_sb[:, 2 * t : 2 * t + 2, :],
                        ps[:].rearrange("p (a b) -> p a b", a=2),
                        mybir.ActivationFunctionType.Identity,
                        bias=bias[:],
                    )
                    # flush finished rows on the ACT HWDGE ring as soon as they
                    # complete; the last block flushes in quarters to shrink
                    # the kernel tail
                    flush_at = (
                        (7, 11, 13, 14) if last else (BH // 4 - 1, BH // 2 - 1)
                    )
                    if t in flush_at:
                        fi = flush_at.index(t)
                        prev = 0 if fi == 0 else (flush_at[fi - 1] + 1)
                        lo, hi = 2 * prev, 2 * t + 2
                        nc.scalar.dma_start(
                            y_d[:, BH * blk + lo : BH * blk + hi, :],
                            out_sb[:, lo:hi, :],
                        )
                if last:
                    for r in (BH - 2, BH - 1):
                        psr = ppool.tile([C, WO], F32, name="psr", tag="ps")
                        for tap in range(K * K):
                            kh, kw = tap // K, tap % K
                            cb = _KW_BASE[kw]
                            nc.tensor.matmul(
                                psr[:],
                                wt[:, tap, :],
                                xt[:, 2 * r + kh, cb : cb + WO],
                                start=(tap == 0),
                                stop=(tap == K * K - 1),
                            )
                        nc.scalar.activation(
                            out_sb[:, r, :],
                            psr[:],
                            mybir.ActivationFunctionType.Identity,
                            bias=bias[:],
                        )
                        nc.scalar.dma_start(
                            y_d[:, BH * blk + r, :], out_sb[:, r, :]
                        )

    nc.compile()
    return nc


def _get_program():
    if "nc" not in _PROGRAM_CACHE:
        _PROGRAM_CACHE["nc"] = _build_program()
    return _PROGRAM_CACHE["nc"]


def _prepare_inputs(x, coeff, basis, bias):
    """Host prep: effective weights, padded phase-split e3m4 x, shards."""
    x = np.asarray(x)
    coeff = np.asarray(coeff)
    basis = np.asarray(basis)
    bias = np.asarray(bias)
    B = coeff.shape[2]
    # wt[i, tap, o] = sum_b coeff[o,i,b] * basis[b, tap], pre-scaled 1/XSCALE
    w_eff = (
        coeff.astype(np.float32).reshape(C * C, B)
        @ basis.astype(np.float32).reshape(B, K * K)
    ).reshape(C, C, K * K)
    wt = np.ascontiguousarray(
        w_eff.transpose(1, 2, 0) * np.float32(1.0 / XSCALE)
    ).astype(np.float16)

    xs = np.clip(x.astype(np.float32) * np.float32(XSCALE), -XCLIP, XCLIP)
    xb = xs.astype(NP_XDT)
    xph = np.zeros((x.shape[0], C, HP, WP), dtype=NP_XDT)
    # even phase: padded col 2j -> orig col 2j-1  (slot j=1..256)
    xph[:, :, 1 : H + 1, 1:257] = xb[:, :, :, 1::2]
    # odd phase: padded col 2j+1 -> orig col 2j  (slot 257+j, j=0..255)
    xph[:, :, 1 : H + 1, 257:513] = xb[:, :, :, 0::2]

    bias2 = np.ascontiguousarray(bias.astype(np.float32).reshape(C, 1))

    in_maps = []
    for s in range(N_CORES):
        b_idx, h_idx = divmod(s, 2)
        shard = np.ascontiguousarray(
            xph[b_idx, :, 256 * h_idx : 256 * h_idx + SH_ROWS, :]
        )
        in_maps.append({"x": shard, "wt": wt, "bias": bias2})
    return in_maps


def _assemble(results, n_batch):
    out = np.empty((n_batch, C, 2 * CORE_HO, WO), dtype=np.float32)
    for s in range(N_CORES):
        b_idx, h_idx = divmod(s, 2)
        out[b_idx, :, CORE_HO * h_idx : CORE_HO * (h_idx + 1), :] = results[s][
            "out"
        ].astype(np.float32)
    return out


def run(x, coeff, basis, bias, trace=False, trace_cores=None):
    """Run the kernel; returns (full_output, BassKernelResults)."""
    nc = _get_program()
    in_maps = _prepare_inputs(x, coeff, basis, bias)
    last_err = None
    for attempt in range(3):
        try:
            res = _bu.run_bass_kernel_spmd(
                nc,
                in_maps,
                list(range(N_CORES)),
                trace=trace,
                trace_cores=trace_cores,
            )
            return _assemble(res.results, x.shape[0]), res
        except Exception as e:  # transient NRT device-unrecoverable after
            last_err = e        # abrupt neighbor-process exits; nudge + retry
            if attempt == 2 or "UNAVAILABLE" not in str(e):
                raise
            import time

            import jax
            import jax.numpy as jnp

            time.sleep(15)
            try:
                a = jnp.ones((8, 8))
                (a @ a).block_until_ready()
            except Exception:
                time.sleep(15)
    raise last_err


def kernel(x, coeff, basis, bias):
    out, _ = run(x, coeff, basis, bias, trace=False)
    return out
